# revision 15
# baseline (speedup 1.0000x reference)
"""CLIP causal attention (B=8, T=1024, E=768, H=12) on 8 TRN2 NeuronCores.

Strategy: pure data-parallel over batch — core b handles x[b] end to end,
no collectives. All compute in transposed space (embed on partitions):

  X' = x_b^T                       [768, 1024]  (host pre-transposed, bf16)
  Q' = Wq^T @ X' (+bq)             [768, 1024]  lhsT = Wq as stored
  K' = Wk^T @ X' (+bk)             [768, 1024]
  V  = X'^T @ Wv (+bv)             [1024, 768]  lhsT = X' blocks (j on partitions)

Attention runs per HEAD PAIR (the two heads sharing a 128-row Q'/K' block).
The two scores matmuls have K=64 contraction -> they occupy disjoint row
groups (partitions 0-63 / 64-127) of the PE array and run CONCURRENTLY
(row tiling), so a pair's scores cost one matmul span instead of two.

Per pair, attention is split into two i-chunk phases (i in [0,512), then
[512,1024)) so PSUM fits: scores tiles [128,1024] (h0|h1, 2 banks, 2 bufs)
+ one O accumulator [128,1024] (h0|h1, 2 banks) + 2 proj banks = 8 banks.

  per (pair, phase ic, j-tile jt):
     S2[:, h*512+d0c:(h+1)*512] = K'_h[:,jblk]^T @ Q'_h[chunk]   (2 MMs, packed)
     P2 = exp(S2 * 1/8)       one ScalarE ACT for both heads (3D AP)
     causal: skip masked blocks, restrict cols, tri-mask the diag block
     o_ps[0:65, h*512+...] += Vaug_h[j,65]^T @ P2_h  (col 64 of Vaug = ones
                                                      -> row 64 = denominator)
  normalize per phase: one [1,1024] denom copy + one reciprocal + one
  gpsimd partition-broadcast covers both heads; 2 DVE muls write O'.

  out = (O'^T @ Wo) + bo           [1024, 768]  lhsT = O' blocks -> direct
                                                untransposed output

Q/K projections interleave with the pairs that consume them; V projection
(two e-chunks) and early out-proj tiles fill PE while ScalarE runs exp.
All matmul operands bf16 (fp32 PSUM accumulation); measured end-to-end
rel l2 err vs fp32 reference ~5e-3.
"""

import numpy as np
import ml_dtypes

E = 768
T = 1024
B = 8
H = 12
DH = 64
NT = E // 128          # 6 partition-tiles of the embed dim
NJ = T // 128          # 8 partition-tiles of the token dim
SCALE = DH ** -0.5     # folded into the exp() activation's scale operand
VW = 128               # V_aug row width per head: 64 data + 64 ones cols
                       # (ones replicated so the denominator comes out of the
                       # AV matmul already partition-replicated: no gpsimd
                       # broadcast in the normalize chain)

_CACHE = {}


def _build():
    import concourse.bass as bass
    import concourse.tile as tile
    from concourse import bacc, mybir

    f32 = mybir.dt.float32
    bf16 = mybir.dt.bfloat16
    Exp = mybir.ActivationFunctionType.Exp

    nc = bacc.Bacc(
        "TRN2",
        target_bir_lowering=False,
        debug=False,
        enable_asserts=False,
        num_devices=B,
    )

    xt = nc.dram_tensor("xt", [E, T], bf16, kind="ExternalInput").ap()
    wq = nc.dram_tensor("wq", [E, E], bf16, kind="ExternalInput").ap()
    wk = nc.dram_tensor("wk", [E, E], bf16, kind="ExternalInput").ap()
    wv = nc.dram_tensor("wv", [E, E], bf16, kind="ExternalInput").ap()
    wo = nc.dram_tensor("wo", [E, E], bf16, kind="ExternalInput").ap()
    bqt = nc.dram_tensor("bqt", [128, NT], f32, kind="ExternalInput").ap()
    bkt = nc.dram_tensor("bkt", [128, NT], f32, kind="ExternalInput").ap()
    bvr = nc.dram_tensor("bvr", [1, E], bf16, kind="ExternalInput").ap()
    bor = nc.dram_tensor("bor", [1, E], bf16, kind="ExternalInput").ap()
    tri = nc.dram_tensor("tri", [128, 256], bf16, kind="ExternalInput").ap()
    out = nc.dram_tensor("out", [T, E], f32, kind="ExternalOutput").ap()

    with tile.TileContext(nc) as tc:
        with (
            tc.tile_pool(name="const", bufs=1) as cpool,
            tc.tile_pool(name="psb", bufs=4) as ppool,
            tc.tile_pool(name="rsb", bufs=4) as rpool,
            tc.tile_pool(name="rbsb", bufs=2) as rbpool,
            tc.tile_pool(name="fin", bufs=3) as fpool,
            tc.tile_pool(name="pp", bufs=2, space="PSUM") as pp,
            tc.tile_pool(name="sp", bufs=2, space="PSUM") as sp,
            tc.tile_pool(name="op", bufs=1, space="PSUM") as op,
        ):
            XT = cpool.tile([128, NT * T], bf16)     # (kt, i)
            WQ = cpool.tile([128, NT * E], bf16)     # (kt, n)
            WK = cpool.tile([128, NT * E], bf16)
            WV = cpool.tile([128, NT * E], bf16)
            WO = cpool.tile([128, NT * E], bf16)
            QS = cpool.tile([128, NT * T], bf16)     # Q' (nt, i)
            KS = cpool.tile([128, NT * T], bf16)
            VS = cpool.tile([128, NJ * H * VW], bf16)  # (jt, h, 64 data + ones)
            OS = cpool.tile([128, NT * T], bf16)     # O' (et, i)
            BQ = cpool.tile([128, NT], f32)
            BK = cpool.tile([128, NT], f32)
            BVR = cpool.tile([1, E], bf16)
            BOR = cpool.tile([1, E], bf16)
            TRI = cpool.tile([128, 256], bf16)  # causal triangle, twice side
                                                # by side (per-head tri-mask
                                                # without stride-0 APs)

            # ---- input DMAs: tiny constants first (Q/K/V evictions need the
            # biases; don't queue them behind 4.5MB of weights), then
            # per-k-tile splits so compute can start early. WV before WQ/WK:
            # the V projection (which only needs XT + WV) runs first. ----
            nc.sync.dma_start(BQ[:], bqt)
            nc.sync.dma_start(BK[:], bkt)
            nc.sync.dma_start(BVR[:], bvr)
            nc.sync.dma_start(BOR[:], bor)
            nc.sync.dma_start(TRI[:], tri)
            BVB = cpool.tile([128, E], bf16)
            FINB = cpool.tile([128, E], bf16)
            nc.gpsimd.partition_broadcast(BVB[:], BVR[:])
            nc.gpsimd.partition_broadcast(FINB[:], BOR[:])
            xt3 = xt.rearrange("(k p) i -> p k i", p=128)
            w3 = {
                id(WQ): wq.rearrange("(k p) n -> p k n", p=128),
                id(WK): wk.rearrange("(k p) n -> p k n", p=128),
                id(WV): wv.rearrange("(k p) n -> p k n", p=128),
                id(WO): wo.rearrange("(k p) n -> p k n", p=128),
            }
            for kt in range(NT):
                nc.sync.dma_start(XT[:, kt * T : (kt + 1) * T], xt3[:, kt])
                nc.sync.dma_start(WV[:, kt * E : (kt + 1) * E], w3[id(WV)][:, kt])
            for W in (WQ, WK, WO):
                for kt in range(NT):
                    nc.sync.dma_start(W[:, kt * E : (kt + 1) * E], w3[id(W)][:, kt])
            # V_aug ones columns via one strided DVE memset. NOT a strided
            # DMA: on a cold first run the 12K tiny writes of that DMA lose
            # the race against pair 0's AV matmuls (HW-only; CoreSim orders
            # it correctly) and poison the softmax denominators.
            nc.vector.memset(
                VS[:].rearrange("p (j h e) -> p j h e", h=H, e=VW)[:, :, :, 64:128],
                1.0,
            )
            # ---- PE warmup: dummy matmuls with no DMA dependency so the
            # HAM activity monitor lifts the 1.2GHz cold gate before real
            # work arrives (DUM memset first: it gates the dummies) ----
            DUMW = cpool.tile([128, 128], bf16)
            DUMR = cpool.tile([128, 512], bf16)
            nc.vector.memset(DUMW[:], 1.0)
            nc.vector.memset(DUMR[:], 1.0)


            def dummy(n=512):
                # full-array junk matmul: the HAM activity monitor only lifts
                # the 1.2GHz cold gate for real array occupancy.
                d_ps = pp.tile([128, 512], f32, tag="proj")
                nc.tensor.matmul(
                    d_ps[:, :n], lhsT=DUMW[:], rhs=DUMR[:, :n], start=True, stop=True
                )

            for _ in range(14):
                dummy()

            # ---- V projection: lhsT = X'[kt, jblk] -> V[j, e] into the
            # 65-col-per-head V_aug layout. Chunk A covers heads 0-7 (needed
            # by pairs 0-3), chunk B heads 8-11 (emitted after qk_proj(0) so
            # it fills pair 0/1 exp-waits without delaying pair 0's start).
            def v_proj(e0, ew, h0, nh):
                for jt in range(NJ):
                    if e0 == 0:
                        # chunk A runs during the input-DMA stream: pad the
                        # chunk-to-chunk DMA waits with junk work so the HAM
                        # activity window never re-throttles the clock
                        dummy()
                    ps = pp.tile([128, 512], f32, tag="proj")
                    for kt in range(NT):
                        nc.tensor.matmul(
                            ps[:, :ew],
                            lhsT=XT[:, kt * T + jt * 128 : kt * T + jt * 128 + 128],
                            rhs=WV[:, kt * E + e0 : kt * E + e0 + ew],
                            start=(kt == 0),
                            stop=(kt == NT - 1),
                        )
                    dst = (
                        VS[:, (jt * H + h0) * VW : (jt * H + h0 + nh) * VW]
                        .rearrange("p (h e) -> p h e", e=VW)[:, :, 0:64]
                    )
                    nc.vector.tensor_add(
                        dst,
                        ps[:, :ew].rearrange("p (h d) -> p h d", d=64),
                        BVB[:, e0 : e0 + ew].rearrange("p (h d) -> p h d", d=64),
                    )

            # ---- Q'/K' projection for one 128-row block nt (2 heads) ----
            def qk_proj(nt):
                for W, Bb, DST in ((WQ, BQ, QS), (WK, BK, KS)):
                    for ic in range(2):
                        ps = pp.tile([128, 512], f32, tag="proj")
                        for kt in range(NT):
                            nc.tensor.matmul(
                                ps[:],
                                lhsT=W[:, kt * E + nt * 128 : kt * E + nt * 128 + 128],
                                rhs=XT[:, kt * T + ic * 512 : kt * T + ic * 512 + 512],
                                start=(kt == 0),
                                stop=(kt == NT - 1),
                            )
                        nc.vector.tensor_scalar_add(
                            DST[:, nt * T + ic * 512 : nt * T + ic * 512 + 512],
                            ps[:],
                            Bb[:, nt : nt + 1],
                        )

            # ---- attention for one head pair (heads 2nt, 2nt+1), KQ
            # orientation (j on partitions, i free). Two phases, one per
            # 512-col i-chunk, so one [128,1024] O accumulator (h0|h1)
            # serves the whole phase and PSUM stays within 8 banks. ----
            def pair(nt):
                for ic in range(2):
                    o_ps = op.tile([128, 1024], f32, tag="oaug")
                    jts = range(4) if ic == 0 else range(NJ)
                    last = jts[-1]
                    for jt in jts:
                        d0c = max(0, jt * 128 - ic * 512)
                        s2 = sp.tile([128, 1024], f32, tag="scores")
                        p2 = ppool.tile([128, 1024], bf16, tag="probs")
                        for h in range(2):
                            # h1 always writes its full 512-chunk: its extra
                            # sub-diagonal cols are valid-but-masked scores
                            # that AV later skips. This keeps the whole exp
                            # span [d0c:1024] initialized so one contiguous 2D
                            # ACT covers both heads.
                            po = h * 64
                            c0 = d0c if h == 0 else 0
                            nc.tensor.matmul(
                                s2[:, h * 512 + c0 : (h + 1) * 512],
                                lhsT=KS[po : po + 64,
                                        nt * T + jt * 128 : nt * T + jt * 128 + 128],
                                rhs=QS[po : po + 64,
                                       nt * T + ic * 512 + c0 : nt * T + (ic + 1) * 512],
                                start=True,
                                stop=True,
                                skip_group_check=True,
                            )
                        # one exp for both heads: a single contiguous 2D span
                        # [d0c:1024]. The middle [512:512+d0c] (h1's masked-out
                        # cols) is exp'd too but never read by the AV matmuls —
                        # cheaper than per-head ACTs, and 2D APs are the
                        # HW-safe path for ScalarE.
                        nc.scalar.activation(
                            p2[:, d0c:1024], s2[:, d0c:1024], Exp, scale=SCALE
                        )
                        if ic * 512 <= jt * 128 < (ic + 1) * 512:
                            # causal triangle on both heads' diag blocks in one
                            # DVE op (3D AP, stride 512; TRI holds the triangle
                            # twice so no stride-0 operand is needed). NOT
                            # gpsimd affine_select: mixing custom-op types on
                            # GpSimd forces MODIFY_POOL_CONFIG switches that
                            # stall partition_broadcast.
                            pd = p2[:].rearrange("p (h w) -> p h w", h=2)[
                                :, :, d0c : d0c + 128
                            ]
                            nc.vector.tensor_mul(
                                pd,
                                pd,
                                TRI[:].rearrange("p (o w) -> p o w", o=2),
                            )
                        for h in range(2):
                            hg = 2 * nt + h
                            nc.tensor.matmul(
                                o_ps[:, h * 512 + d0c : (h + 1) * 512],
                                lhsT=VS[:, (jt * H + hg) * VW : (jt * H + hg + 1) * VW],
                                rhs=p2[:, h * 512 + d0c : (h + 1) * 512],
                                start=(jt == 0),
                                stop=(jt == last),
                                skip_group_check=True,
                            )
                    # normalize the phase: softmax denominators live in rows
                    # 64-127 (the replicated V_aug ones cols) of both halves,
                    # already broadcast across 64 partitions by the matmul
                    # itself. Copy to SBUF (recip needs an SBUF operand),
                    # reciprocal, then scale both heads.
                    dn = rbpool.tile([64, 1024], f32, tag="denom")
                    nc.vector.tensor_copy(dn[:], o_ps[64:128, :])
                    rb = rbpool.tile([64, 1024], f32, tag="recipb")
                    nc.vector.reciprocal_approx_fast(rb[:], dn[:])
                    for h in range(2):
                        nc.vector.tensor_mul(
                            OS[h * 64 : (h + 1) * 64,
                               nt * T + ic * 512 : nt * T + (ic + 1) * 512],
                            o_ps[0:64, h * 512 : (h + 1) * 512],
                            rb[0:64, h * 512 : (h + 1) * 512],
                        )
                    if nt >= 2:
                        # the proj filler pool is thin from here on: junk
                        # matmuls bridge the normalize drain so the HAM MID
                        # window never sees >3.4us of PE idle (re-throttle to
                        # 1.2GHz costs far more than the junk work)
                        dummy()
                        dummy()
                        dummy()

            # ---- output projection for one row block it of the output ----
            def out_proj(it):
                fin = fpool.tile([128, E], f32, tag="fin")
                for n0, nw in ((0, 512), (512, 256)):
                    f_ps = pp.tile([128, 512], f32, tag="proj")
                    for et in range(NT):
                        nc.tensor.matmul(
                            f_ps[:, :nw],
                            lhsT=OS[:, et * T + it * 128 : et * T + it * 128 + 128],
                            rhs=WO[:, et * E + n0 : et * E + n0 + nw],
                            start=(et == 0),
                            stop=(et == NT - 1),
                        )
                    nc.vector.tensor_add(
                        fin[:, n0 : n0 + nw], f_ps[:, :nw], FINB[:, n0 : n0 + nw]
                    )
                    nc.sync.dma_start(
                        out[it * 128 : (it + 1) * 128, n0 : n0 + nw],
                        fin[:, n0 : n0 + nw],
                    )

            # Interleave: each nt's Q/K projection feeds its pair; the next
            # nt's projection matmuls keep PE busy while ScalarE runs this
            # pair's exps. V chunk B and the first two out-proj tiles are
            # placed to fill the otherwise proj-less late-pair exp waits.
            v_proj(0, 512, 0, 8)
            qk_proj(0)
            v_proj(512, 256, 8, 4)
            for nt in range(NT):
                pair(nt)
                if nt + 1 < NT:
                    qk_proj(nt + 1)
            for it in range(NJ):
                out_proj(it)

    nc.compile()
    return nc


def _get_nc():
    if "nc" not in _CACHE:
        _CACHE["nc"] = _build()
    return _CACHE["nc"]


def _make_in_maps(inputs):
    bf = ml_dtypes.bfloat16
    x = np.asarray(inputs["x"], np.float32)
    shared = {
        "wq": np.asarray(inputs["Wq"], np.float32).astype(bf),
        "wk": np.asarray(inputs["Wk"], np.float32).astype(bf),
        "wv": np.asarray(inputs["Wv"], np.float32).astype(bf),
        "wo": np.asarray(inputs["Wo"], np.float32).astype(bf),
        "bqt": np.ascontiguousarray(
            np.asarray(inputs["bq"], np.float32).reshape(NT, 128).T
        ),
        "bkt": np.ascontiguousarray(
            np.asarray(inputs["bk"], np.float32).reshape(NT, 128).T
        ),
        "bvr": np.asarray(inputs["bv"], np.float32).reshape(1, E).astype(bf),
        "bor": np.asarray(inputs["bo"], np.float32).reshape(1, E).astype(bf),
        "tri": np.tile(np.triu(np.ones((128, 128), np.float32)), (1, 2)).astype(bf),
    }
    return [dict(shared, xt=x[b].T.astype(bf)) for b in range(B)]


def _run(inputs, trace=False):
    from concourse import bass_utils

    nc = _get_nc()
    res = bass_utils.run_bass_kernel_spmd(
        nc, _make_in_maps(inputs), core_ids=list(range(B)), trace=trace
    )
    out = np.stack([np.asarray(res.results[c]["out"]) for c in range(B)])
    return out, res


def kernel(**inputs) -> np.ndarray:
    out, _ = _run(inputs, trace=False)
    return out


# revision 16
# speedup vs baseline: 1.0325x; 1.0325x over previous
"""CLIP causal attention (B=8, T=1024, E=768, H=12) on 8 TRN2 NeuronCores.

Strategy: pure data-parallel over batch — core b handles x[b] end to end,
no collectives. All compute in transposed space (embed on partitions):

  X' = x_b^T                       [768, 1024]  (host pre-transposed, bf16)
  Q' = Wq^T @ X' (+bq)             [768, 1024]  lhsT = Wq as stored
  K' = Wk^T @ X' (+bk)             [768, 1024]
  V  = X'^T @ Wv (+bv)             [1024, 768]  lhsT = X' blocks (j on partitions)

Attention runs per HEAD PAIR (the two heads sharing a 128-row Q'/K' block).
The two scores matmuls have K=64 contraction -> they occupy disjoint row
groups (partitions 0-63 / 64-127) of the PE array and run CONCURRENTLY
(row tiling), so a pair's scores cost one matmul span instead of two.

Per pair, attention is split into two i-chunk phases (i in [0,512), then
[512,1024)) so PSUM fits: scores tiles [128,1024] (h0|h1, 2 banks, 2 bufs)
+ one O accumulator [128,1024] (h0|h1, 2 banks) + 2 proj banks = 8 banks.

  per (pair, phase ic, j-tile jt):
     S2[:, h*512+d0c:(h+1)*512] = K'_h[:,jblk]^T @ Q'_h[chunk]   (2 MMs, packed)
     P2 = exp(S2 * 1/8)       one ScalarE ACT for both heads (3D AP)
     causal: skip masked blocks, restrict cols, tri-mask the diag block
     o_ps[0:65, h*512+...] += Vaug_h[j,65]^T @ P2_h  (col 64 of Vaug = ones
                                                      -> row 64 = denominator)
  normalize per phase: one [1,1024] denom copy + one reciprocal + one
  gpsimd partition-broadcast covers both heads; 2 DVE muls write O'.

  out = (O'^T @ Wo) + bo           [1024, 768]  lhsT = O' blocks -> direct
                                                untransposed output

Q/K projections interleave with the pairs that consume them; V projection
(two e-chunks) and early out-proj tiles fill PE while ScalarE runs exp.
All matmul operands bf16 (fp32 PSUM accumulation); measured end-to-end
rel l2 err vs fp32 reference ~5e-3.
"""

import numpy as np
import ml_dtypes

E = 768
T = 1024
B = 8
H = 12
DH = 64
NT = E // 128          # 6 partition-tiles of the embed dim
NJ = T // 128          # 8 partition-tiles of the token dim
SCALE = DH ** -0.5     # folded into the exp() activation's scale operand
VW = 128               # V_aug row width per head: 64 data + 64 ones cols
                       # (ones replicated so the denominator comes out of the
                       # AV matmul already partition-replicated: no gpsimd
                       # broadcast in the normalize chain)

_CACHE = {}


def _build():
    import concourse.bass as bass
    import concourse.tile as tile
    from concourse import bacc, mybir

    f32 = mybir.dt.float32
    bf16 = mybir.dt.bfloat16
    Exp = mybir.ActivationFunctionType.Exp

    nc = bacc.Bacc(
        "TRN2",
        target_bir_lowering=False,
        debug=False,
        enable_asserts=False,
        num_devices=B,
    )

    xt = nc.dram_tensor("xt", [E, T], bf16, kind="ExternalInput").ap()
    wq = nc.dram_tensor("wq", [E, E], bf16, kind="ExternalInput").ap()
    wk = nc.dram_tensor("wk", [E, E], bf16, kind="ExternalInput").ap()
    wv = nc.dram_tensor("wv", [E, E], bf16, kind="ExternalInput").ap()
    wo = nc.dram_tensor("wo", [E, E], bf16, kind="ExternalInput").ap()
    bqt = nc.dram_tensor("bqt", [128, NT], f32, kind="ExternalInput").ap()
    bkt = nc.dram_tensor("bkt", [128, NT], f32, kind="ExternalInput").ap()
    bvr = nc.dram_tensor("bvr", [1, E], bf16, kind="ExternalInput").ap()
    bor = nc.dram_tensor("bor", [1, E], bf16, kind="ExternalInput").ap()
    tri = nc.dram_tensor("tri", [128, 256], bf16, kind="ExternalInput").ap()
    out = nc.dram_tensor("out", [T, E], f32, kind="ExternalOutput").ap()

    with tile.TileContext(nc) as tc:
        with (
            tc.tile_pool(name="const", bufs=1) as cpool,
            tc.tile_pool(name="psb", bufs=4) as ppool,
            tc.tile_pool(name="rsb", bufs=4) as rpool,
            tc.tile_pool(name="rbsb", bufs=2) as rbpool,
            tc.tile_pool(name="fin", bufs=3) as fpool,
            tc.tile_pool(name="pp", bufs=2, space="PSUM") as pp,
            tc.tile_pool(name="sp", bufs=2, space="PSUM") as sp,
            tc.tile_pool(name="op", bufs=1, space="PSUM") as op,
        ):
            XT = cpool.tile([128, NT * T], bf16)     # (kt, i)
            WQ = cpool.tile([128, NT * E], bf16)     # (kt, n)
            WK = cpool.tile([128, NT * E], bf16)
            WV = cpool.tile([128, NT * E], bf16)
            WO = cpool.tile([128, NT * E], bf16)
            QS = cpool.tile([128, NT * T], bf16)     # Q' (nt, i)
            KS = cpool.tile([128, NT * T], bf16)
            VS = cpool.tile([128, NJ * H * VW], bf16)  # (jt, h, 64 data + ones)
            OS = cpool.tile([128, NT * T], bf16)     # O' (et, i)
            BQ = cpool.tile([128, NT], f32)
            BK = cpool.tile([128, NT], f32)
            BVR = cpool.tile([1, E], bf16)
            BOR = cpool.tile([1, E], bf16)
            TRI = cpool.tile([128, 256], bf16)  # causal triangle, twice side
                                                # by side (per-head tri-mask
                                                # without stride-0 APs)

            # ---- input DMAs: tiny constants first (Q/K/V evictions need the
            # biases; don't queue them behind 4.5MB of weights), then
            # per-k-tile splits so compute can start early. WV before WQ/WK:
            # the V projection (which only needs XT + WV) runs first. ----
            nc.sync.dma_start(BQ[:], bqt)
            nc.sync.dma_start(BK[:], bkt)
            nc.sync.dma_start(BVR[:], bvr)
            nc.sync.dma_start(BOR[:], bor)
            nc.sync.dma_start(TRI[:], tri)
            BVB = cpool.tile([128, E], bf16)
            FINB = cpool.tile([128, E], bf16)
            nc.gpsimd.partition_broadcast(BVB[:], BVR[:])
            nc.gpsimd.partition_broadcast(FINB[:], BOR[:])
            xt3 = xt.rearrange("(k p) i -> p k i", p=128)
            w3 = {
                id(WQ): wq.rearrange("(k p) n -> p k n", p=128),
                id(WK): wk.rearrange("(k p) n -> p k n", p=128),
                id(WV): wv.rearrange("(k p) n -> p k n", p=128),
                id(WO): wo.rearrange("(k p) n -> p k n", p=128),
            }
            for kt in range(NT):
                nc.sync.dma_start(XT[:, kt * T : (kt + 1) * T], xt3[:, kt])
                nc.sync.dma_start(WV[:, kt * E : (kt + 1) * E], w3[id(WV)][:, kt])
            for W in (WQ, WK, WO):
                for kt in range(NT):
                    nc.sync.dma_start(W[:, kt * E : (kt + 1) * E], w3[id(W)][:, kt])
            # V_aug ones columns via one strided DVE memset. NOT a strided
            # DMA: on a cold first run the 12K tiny writes of that DMA lose
            # the race against pair 0's AV matmuls (HW-only; CoreSim orders
            # it correctly) and poison the softmax denominators.
            nc.vector.memset(
                VS[:].rearrange("p (j h e) -> p j h e", h=H, e=VW)[:, :, :, 64:128],
                1.0,
            )
            # ---- PE warmup: dummy matmuls with no DMA dependency so the
            # HAM activity monitor lifts the 1.2GHz cold gate before real
            # work arrives (DUM memset first: it gates the dummies) ----
            DUMW = cpool.tile([128, 128], bf16)
            DUMR = cpool.tile([128, 512], bf16)
            nc.vector.memset(DUMW[:], 1.0)
            nc.vector.memset(DUMR[:], 1.0)


            def dummy(n=512):
                # full-array junk matmul: the HAM activity monitor only lifts
                # the 1.2GHz cold gate for real array occupancy.
                d_ps = pp.tile([128, 512], f32, tag="proj")
                nc.tensor.matmul(
                    d_ps[:, :n], lhsT=DUMW[:], rhs=DUMR[:, :n], start=True, stop=True
                )

            for _ in range(14):
                dummy()

            # ---- V projection: lhsT = X'[kt, jblk] -> V[j, e] into the
            # 65-col-per-head V_aug layout. Chunk A covers heads 0-7 (needed
            # by pairs 0-3), chunk B heads 8-11 (emitted after qk_proj(0) so
            # it fills pair 0/1 exp-waits without delaying pair 0's start).
            def v_proj(e0, ew, h0, nh):
                for jt in range(NJ):
                    ps = pp.tile([128, 512], f32, tag="proj")
                    for kt in range(NT):
                        nc.tensor.matmul(
                            ps[:, :ew],
                            lhsT=XT[:, kt * T + jt * 128 : kt * T + jt * 128 + 128],
                            rhs=WV[:, kt * E + e0 : kt * E + e0 + ew],
                            start=(kt == 0),
                            stop=(kt == NT - 1),
                        )
                    dst = (
                        VS[:, (jt * H + h0) * VW : (jt * H + h0 + nh) * VW]
                        .rearrange("p (h e) -> p h e", e=VW)[:, :, 0:64]
                    )
                    nc.vector.tensor_add(
                        dst,
                        ps[:, :ew].rearrange("p (h d) -> p h d", d=64),
                        BVB[:, e0 : e0 + ew].rearrange("p (h d) -> p h d", d=64),
                    )

            # ---- Q'/K' projection for one 128-row block nt (2 heads) ----
            def qk_proj(nt):
                for W, Bb, DST in ((WQ, BQ, QS), (WK, BK, KS)):
                    for ic in range(2):
                        ps = pp.tile([128, 512], f32, tag="proj")
                        for kt in range(NT):
                            nc.tensor.matmul(
                                ps[:],
                                lhsT=W[:, kt * E + nt * 128 : kt * E + nt * 128 + 128],
                                rhs=XT[:, kt * T + ic * 512 : kt * T + ic * 512 + 512],
                                start=(kt == 0),
                                stop=(kt == NT - 1),
                            )
                        nc.vector.tensor_scalar_add(
                            DST[:, nt * T + ic * 512 : nt * T + ic * 512 + 512],
                            ps[:],
                            Bb[:, nt : nt + 1],
                        )

            # ---- attention for one head pair (heads 2nt, 2nt+1), KQ
            # orientation (j on partitions, i free). Two phases, one per
            # 512-col i-chunk, so one [128,1024] O accumulator (h0|h1)
            # serves the whole phase and PSUM stays within 8 banks. ----
            def pair(nt):
                for ic in range(2):
                    o_ps = op.tile([128, 1024], f32, tag="oaug")
                    jts = range(4) if ic == 0 else range(NJ)
                    last = jts[-1]
                    for jt in jts:
                        d0c = max(0, jt * 128 - ic * 512)
                        s2 = sp.tile([128, 1024], f32, tag="scores")
                        p2 = ppool.tile([128, 1024], bf16, tag="probs")
                        for h in range(2):
                            # h1 always writes its full 512-chunk: its extra
                            # sub-diagonal cols are valid-but-masked scores
                            # that AV later skips. This keeps the whole exp
                            # span [d0c:1024] initialized so one contiguous 2D
                            # ACT covers both heads.
                            po = h * 64
                            c0 = d0c if h == 0 else 0
                            nc.tensor.matmul(
                                s2[:, h * 512 + c0 : (h + 1) * 512],
                                lhsT=KS[po : po + 64,
                                        nt * T + jt * 128 : nt * T + jt * 128 + 128],
                                rhs=QS[po : po + 64,
                                       nt * T + ic * 512 + c0 : nt * T + (ic + 1) * 512],
                                start=True,
                                stop=True,
                                skip_group_check=True,
                            )
                        # one exp for both heads: a single contiguous 2D span
                        # [d0c:1024]. The middle [512:512+d0c] (h1's masked-out
                        # cols) is exp'd too but never read by the AV matmuls —
                        # cheaper than per-head ACTs, and 2D APs are the
                        # HW-safe path for ScalarE.
                        nc.scalar.activation(
                            p2[:, d0c:1024], s2[:, d0c:1024], Exp, scale=SCALE
                        )
                        if ic * 512 <= jt * 128 < (ic + 1) * 512:
                            # causal triangle on both heads' diag blocks in one
                            # DVE op (3D AP, stride 512; TRI holds the triangle
                            # twice so no stride-0 operand is needed). NOT
                            # gpsimd affine_select: mixing custom-op types on
                            # GpSimd forces MODIFY_POOL_CONFIG switches that
                            # stall partition_broadcast.
                            pd = p2[:].rearrange("p (h w) -> p h w", h=2)[
                                :, :, d0c : d0c + 128
                            ]
                            nc.vector.tensor_mul(
                                pd,
                                pd,
                                TRI[:].rearrange("p (o w) -> p o w", o=2),
                            )
                        for h in range(2):
                            hg = 2 * nt + h
                            nc.tensor.matmul(
                                o_ps[:, h * 512 + d0c : (h + 1) * 512],
                                lhsT=VS[:, (jt * H + hg) * VW : (jt * H + hg + 1) * VW],
                                rhs=p2[:, h * 512 + d0c : (h + 1) * 512],
                                start=(jt == 0),
                                stop=(jt == last),
                                skip_group_check=True,
                            )
                    # normalize the phase: softmax denominators live in rows
                    # 64-127 (the replicated V_aug ones cols) of both halves,
                    # already broadcast across 64 partitions by the matmul
                    # itself. Copy to SBUF (recip needs an SBUF operand),
                    # reciprocal, then scale both heads.
                    dn = rbpool.tile([64, 1024], f32, tag="denom")
                    nc.vector.tensor_copy(dn[:], o_ps[64:128, :])
                    rb = rbpool.tile([64, 1024], f32, tag="recipb")
                    nc.vector.reciprocal_approx_fast(rb[:], dn[:])
                    for h in range(2):
                        nc.vector.tensor_mul(
                            OS[h * 64 : (h + 1) * 64,
                               nt * T + ic * 512 : nt * T + (ic + 1) * 512],
                            o_ps[0:64, h * 512 : (h + 1) * 512],
                            rb[0:64, h * 512 : (h + 1) * 512],
                        )
                    if nt >= 2:
                        # the proj filler pool is thin from here on: two junk
                        # matmuls bridge the normalize drain so the HAM MID
                        # window never sees >3.4us of PE idle (re-throttle to
                        # 1.2GHz costs far more than the junk work)
                        dummy()
                        dummy()

            # ---- output projection for one row block it of the output ----
            def out_proj(it):
                fin = fpool.tile([128, E], f32, tag="fin")
                for n0, nw in ((0, 512), (512, 256)):
                    f_ps = pp.tile([128, 512], f32, tag="proj")
                    for et in range(NT):
                        nc.tensor.matmul(
                            f_ps[:, :nw],
                            lhsT=OS[:, et * T + it * 128 : et * T + it * 128 + 128],
                            rhs=WO[:, et * E + n0 : et * E + n0 + nw],
                            start=(et == 0),
                            stop=(et == NT - 1),
                        )
                    nc.vector.tensor_add(
                        fin[:, n0 : n0 + nw], f_ps[:, :nw], FINB[:, n0 : n0 + nw]
                    )
                    nc.sync.dma_start(
                        out[it * 128 : (it + 1) * 128, n0 : n0 + nw],
                        fin[:, n0 : n0 + nw],
                    )

            # Interleave: each nt's Q/K projection feeds its pair; the next
            # nt's projection matmuls keep PE busy while ScalarE runs this
            # pair's exps. V chunk B and the first two out-proj tiles are
            # placed to fill the otherwise proj-less late-pair exp waits.
            v_proj(0, 512, 0, 8)
            qk_proj(0)
            v_proj(512, 256, 8, 4)
            for nt in range(NT):
                pair(nt)
                if nt + 1 < NT:
                    qk_proj(nt + 1)
            for it in range(NJ):
                out_proj(it)

    nc.compile()
    return nc


def _get_nc():
    if "nc" not in _CACHE:
        _CACHE["nc"] = _build()
    return _CACHE["nc"]


def _make_in_maps(inputs):
    bf = ml_dtypes.bfloat16
    x = np.asarray(inputs["x"], np.float32)
    shared = {
        "wq": np.asarray(inputs["Wq"], np.float32).astype(bf),
        "wk": np.asarray(inputs["Wk"], np.float32).astype(bf),
        "wv": np.asarray(inputs["Wv"], np.float32).astype(bf),
        "wo": np.asarray(inputs["Wo"], np.float32).astype(bf),
        "bqt": np.ascontiguousarray(
            np.asarray(inputs["bq"], np.float32).reshape(NT, 128).T
        ),
        "bkt": np.ascontiguousarray(
            np.asarray(inputs["bk"], np.float32).reshape(NT, 128).T
        ),
        "bvr": np.asarray(inputs["bv"], np.float32).reshape(1, E).astype(bf),
        "bor": np.asarray(inputs["bo"], np.float32).reshape(1, E).astype(bf),
        "tri": np.tile(np.triu(np.ones((128, 128), np.float32)), (1, 2)).astype(bf),
    }
    return [dict(shared, xt=x[b].T.astype(bf)) for b in range(B)]


def _run(inputs, trace=False):
    from concourse import bass_utils

    nc = _get_nc()
    res = bass_utils.run_bass_kernel_spmd(
        nc, _make_in_maps(inputs), core_ids=list(range(B)), trace=trace
    )
    out = np.stack([np.asarray(res.results[c]["out"]) for c in range(B)])
    return out, res


def kernel(**inputs) -> np.ndarray:
    out, _ = _run(inputs, trace=False)
    return out
